# revision 1
# baseline (speedup 1.0000x reference)
"""Causal multi-head self-attention on 8 Trainium2 NeuronCores.

Sharding: head-parallel. Each of the 8 cores owns 2 of the 16 heads:
it computes Q/K/V for its heads (full sequence), runs causal flash
attention for them entirely on-chip, applies its slice of the output
projection, and writes a full-shape partial output. The host sums the
8 partials.

Layout strategy (no on-device transposes):
  - x is cast to bf16 on host; x^T tiles (d on partitions) are loaded
    via the DMA xbar transpose engine.
  - Q^T, K^T are produced as (128 = [h0|h1] x 64) x t, which is exactly
    the layout the score matmuls need (lhsT = K^T block, rhs = Q^T).
  - Scores are computed transposed, S^T = (k x q), two heads row-packed
    into the two halves of the PE array, landing in adjacent PSUM banks.
  - exp runs on ScalarE straight out of PSUM (scale=1/8 fused), one
    (128 x 1024) call per k-block covering both heads.
  - V is computed directly in (k x dh) layout (x^T block stationary)
    with a 65th all-ones column, so the AV matmul accumulates both the
    attention output and the softmax denominator (row 64) in one pass.
  - Causal masking: full-width blocks everywhere; after exp, the
    lower-left (in k,q space: k>q) region of diagonal blocks is zeroed
    (memset for whole masked sub-blocks, a precomputed 0/1 triangle
    multiply for the boundary sub-block). Zeros flow through AV and the
    denominator untouched.
  - Normalization happens at AV eviction: 1/denominator broadcast over
    partitions (GpSimd) then one fused multiply PSUM->SBUF into ctx^T.
  - Output projection consumes ctx^T blocks as stationary operands so
    the result lands (t x e) and needs no further reshuffling.
"""

import numpy as np
import sys

for _p in ("/opt/trn_rl_repo", "/root/.axon_site/_ro/trn_rl_repo"):
    if _p not in sys.path:
        sys.path.append(_p)

import ml_dtypes

B = 2
S = 4096
D = 1024
H = 16
DH = 64
N_CORES = 8
HEADS_PER_CORE = H // N_CORES  # 2

BF16 = None  # set after import
_cache = {}


def _build(nc, b, s):
    import concourse.bass as bass
    import concourse.mybir as mybir
    from concourse.tile import TileContext
    from contextlib import ExitStack

    dt = mybir.dt
    AF = mybir.ActivationFunctionType
    ALU = mybir.AluOpType

    t_total = b * s          # 8192
    TT = 512                 # t tile (QKV free dim)
    n_ttiles = t_total // TT
    n_dblk = D // 128        # 8
    QT = 512                 # q tile
    n_qt = s // QT           # per batch
    KB = 128                 # k block
    scale = 1.0 / np.sqrt(DH)

    x_d = nc.dram_tensor("xT", [D, t_total], dt.bfloat16, kind="ExternalInput")
    wqkv_d = nc.dram_tensor("wqkvT", [n_dblk, 128, 3 * 128], dt.bfloat16,
                            kind="ExternalInput")
    wout_d = nc.dram_tensor("woutT", [128, D], dt.bfloat16, kind="ExternalInput")
    out_d = nc.dram_tensor("partial_out", [t_total, D], dt.bfloat16,
                           kind="ExternalOutput")

    with TileContext(nc) as tc, ExitStack() as ctx:
        const = ctx.enter_context(tc.tile_pool(name="const", bufs=1))
        # persistent SBUF arrays
        wqkvT = const.tile([128, n_dblk, 3 * 128], dt.bfloat16, tag="wqkv")
        woutT = const.tile([128, D], dt.bfloat16, tag="wout")
        qT = const.tile([128, t_total], dt.bfloat16, tag="qT")
        kT = const.tile([128, t_total], dt.bfloat16, tag="kT")
        # V with ones column: per (batch, head, kblock): (128 x 65)
        n_kblk = s // KB  # 32
        v65 = const.tile([128, b, HEADS_PER_CORE, n_kblk, DH + 1], dt.bfloat16,
                         tag="v65")
        ctxT = const.tile([128, t_total], dt.bfloat16, tag="ctxT")
        tri = const.tile([128, 128], dt.bfloat16, tag="tri")

        ident = const.tile([128, 128], dt.bfloat16, tag="ident")

        nc.sync.dma_start(wqkvT[:], wqkv_d.rearrange("k p e -> p k e"))
        nc.sync.dma_start(woutT[:], wout_d[:])

        # ones column of v65, the 0/1 lower-triangle mask (keep k<=q,
        # i.e. in (k=partition r, q=col c) space keep c >= r), and the
        # identity used by the PE transpose of V.
        nc.vector.memset(v65[:, :, :, :, DH], 1.0)
        nc.gpsimd.memset(tri[:], 1.0)
        nc.gpsimd.affine_select(
            tri[:], tri[:], pattern=[[1, 128]], compare_op=ALU.is_ge,
            fill=0.0, base=0, channel_multiplier=-1,
        )
        nc.gpsimd.affine_select(
            ident[:], tri[:], pattern=[[1, 128]], compare_op=ALU.is_equal,
            fill=0.0, base=0, channel_multiplier=-1,
        )

        xt_pool = ctx.enter_context(tc.tile_pool(name="xt", bufs=12))
        # PSUM budget is 8 banks. Outer (whole-kernel): scores 2-bank
        # tiles x2 bufs = 4 banks, two 1-bank AV accumulators = 2 banks.
        # The remaining 2 banks are time-shared by short-lived inner
        # pools (QKV projection / output projection) below.
        sc_ps = ctx.enter_context(tc.tile_pool(name="sc_ps", bufs=2, space="PSUM"))
        o65_ps = ctx.enter_context(tc.tile_pool(name="o65_ps", bufs=1, space="PSUM"))
        pt_pool = ctx.enter_context(tc.tile_pool(name="pt", bufs=6))
        vt_pool = ctx.enter_context(tc.tile_pool(name="vt", bufs=2))
        ev_pool = ctx.enter_context(tc.tile_pool(name="ev", bufs=4))
        out_sb_pool = ctx.enter_context(tc.tile_pool(name="out_sb", bufs=3))

        def qkv_ttile(qkv_ps, tt):
            """QKV projection for t-range [tt*TT, (tt+1)*TT).

            The pool has 2 one-bank slots; q and k accumulate in the two
            slots, then v reuses q's slot after eviction.
            """
            t0 = tt * TT
            ps_q = qkv_ps.tile([128, TT], dt.float32, tag="qkv")
            ps_k = qkv_ps.tile([128, TT], dt.float32, tag="qkv")
            xts = []
            for dd in range(n_dblk):
                xt = xt_pool.tile([128, TT], dt.bfloat16, tag="xt")
                nc.sync.dma_start(
                    xt[:], x_d[dd * 128:(dd + 1) * 128, t0:t0 + TT])
                xts.append(xt)
                st = dict(start=(dd == 0), stop=(dd == n_dblk - 1))
                nc.tensor.matmul(ps_q[:], wqkvT[:, dd, 0:128], xt[:], **st)
                nc.tensor.matmul(ps_k[:], wqkvT[:, dd, 128:256], xt[:], **st)
            nc.vector.tensor_copy(qT[:, t0:t0 + TT], ps_q[:])
            nc.vector.tensor_copy(kT[:, t0:t0 + TT], ps_k[:])
            ps_vt = qkv_ps.tile([128, TT], dt.float32, tag="qkv")
            for dd in range(n_dblk):
                st = dict(start=(dd == 0), stop=(dd == n_dblk - 1))
                nc.tensor.matmul(ps_vt[:], wqkvT[:, dd, 256:384],
                                 xts[dd][:], **st)
            vt = vt_pool.tile([128, TT], dt.bfloat16, tag="vt")
            nc.vector.tensor_copy(vt[:], ps_vt[:])
            # V^T (e x t) -> V (t x e) via PE transpose, 128x128 blocks
            ps_tv = qkv_ps.tile([128, TT], dt.bfloat16, tag="qkv")
            for j in range(TT // 128):
                nc.tensor.transpose(ps_tv[:, j * 128:(j + 1) * 128],
                                    vt[:, j * 128:(j + 1) * 128], ident[:])
            # scatter V into v65 blocks (t-sub j -> kblock, per head)
            bb = t0 // s
            for j in range(TT // 128):
                kb = (t0 % s) // KB + j
                for h in range(HEADS_PER_CORE):
                    nc.vector.tensor_copy(
                        v65[:, bb, h, kb, 0:DH],
                        ps_tv[:, j * 128 + h * DH: j * 128 + (h + 1) * DH])

        def attention(bb, qt):
            """One q-tile of causal attention for both heads of batch bb.

            All matmuls are issued as row-group-disjoint pairs (two heads
            for scores, two contraction halves for AV) so they dual-issue
            on the PE. Diagonal blocks are narrowed to the valid q range.
            """
            tq0 = bb * s + qt * QT
            o65_h0 = o65_ps.tile([DH + 1, QT], dt.float32, tag="o65h0")
            o65_h1 = o65_ps.tile([DH + 1, QT], dt.float32, tag="o65h1")
            nkb = (qt + 1) * QT // KB
            for kb in range(nkb):
                tk0 = bb * s + kb * KB
                j = kb - qt * (QT // KB)  # >= 0 on the diagonal
                qc0 = max(j, 0) * KB      # first valid local q column
                w = QT - qc0
                ps_s = sc_ps.tile([128, 2 * QT], dt.float32, tag="ps_s")
                nc.tensor.matmul(ps_s[:, 0:w], kT[0:64, tk0:tk0 + KB],
                                 qT[0:64, tq0 + qc0:tq0 + QT],
                                 tile_position=(0, 0))
                nc.tensor.matmul(ps_s[:, QT:QT + w], kT[64:128, tk0:tk0 + KB],
                                 qT[64:128, tq0 + qc0:tq0 + QT],
                                 tile_position=(64, 0))
                pt = pt_pool.tile([128, 2 * QT], dt.bfloat16, tag="pt")
                if j < 0:
                    nc.scalar.activation(pt[:], ps_s[:], AF.Exp, scale=scale)
                else:
                    nc.scalar.activation(pt[:, 0:w], ps_s[:, 0:w],
                                         AF.Exp, scale=scale)
                    nc.scalar.activation(pt[:, QT:QT + w], ps_s[:, QT:QT + w],
                                         AF.Exp, scale=scale)
                    # triangle mask on the first 128 valid columns
                    for half in (0, QT):
                        nc.vector.tensor_tensor(
                            pt[:, half:half + KB], pt[:, half:half + KB],
                            tri[:], ALU.mult)
                st = dict(start=(kb == 0), stop=(kb == nkb - 1))
                nc.tensor.matmul(o65_h0[:, qc0:QT], v65[:, bb, 0, kb, :],
                                 pt[:, 0:w], **st)
                nc.tensor.matmul(o65_h1[:, qc0:QT], v65[:, bb, 1, kb, :],
                                 pt[:, QT:QT + w], **st)
            for h, o65 in ((0, o65_h0), (1, o65_h1)):
                row = ev_pool.tile([1, QT], dt.float32, tag="row")
                rec = ev_pool.tile([1, QT], dt.float32, tag="rec")
                bc = ev_pool.tile([64, QT], dt.float32, tag="bc")
                nc.vector.tensor_copy(row[:], o65[DH:DH + 1, :])
                nc.vector.reciprocal_approx_fast(rec[:], row[:])
                nc.gpsimd.partition_broadcast(bc[:], rec[:])
                nc.vector.tensor_tensor(
                    ctxT[h * DH:(h + 1) * DH, tq0:tq0 + QT],
                    o65[0:DH, :], bc[:], ALU.mult)

        def outproj(out_ps, tb):
            """Output projection for t-block [tb*128, (tb+1)*128)."""
            t0 = tb * 128
            for e in range(D // 512):
                ps = out_ps.tile([128, 512], dt.float32, tag="qkv")
                nc.tensor.matmul(ps[:], ctxT[:, t0:t0 + 128],
                                 woutT[:, e * 512:(e + 1) * 512])
                ob = out_sb_pool.tile([128, 512], dt.bfloat16, tag="ob")
                if (tb + e) % 2 == 0:
                    nc.vector.tensor_copy(ob[:], ps[:])
                else:
                    nc.scalar.copy(ob[:], ps[:])
                nc.sync.dma_start(
                    out_d[t0:t0 + 128, e * 512:(e + 1) * 512], ob[:])

        # Emission order: QKV t-tiles interleaved with the attention
        # q-tiles they unblock (attention qt needs K/V tiles 0..qt),
        # batches interleaved so the ACT-bound attention stream always
        # has PE-side QKV work nearby; output projection at the end.
        tiles_per_batch = n_ttiles // b  # == n_qt
        with tc.tile_pool(name="qkv_ps", bufs=2, space="PSUM") as qkv_ps:
            for tt in range(tiles_per_batch):
                for bb in range(b):
                    qkv_ttile(qkv_ps, bb * tiles_per_batch + tt)
                for bb in range(b):
                    attention(bb, tt)
        with tc.tile_pool(name="out_ps", bufs=2, space="PSUM") as out_ps:
            for tb in range(t_total // 128):
                outproj(out_ps, tb)

    return nc


def _get_kernel(b, s):
    key = (b, s)
    if key not in _cache:
        from concourse import bacc
        nc = bacc.Bacc()
        _build(nc, b, s)
        nc.finalize()  # Bacc compile: reg alloc, library/act-table loads
        _cache[key] = nc
    return _cache[key]


def _prep_inputs(x, Wqkv, Wout):
    """Host-side shard + transpose + bf16 cast. Returns list of in_maps."""
    b, s, d = x.shape
    xT = np.ascontiguousarray(
        x.reshape(b * s, d).astype(ml_dtypes.bfloat16).T)  # (d, b*s)
    n_dblk = d // 128
    in_maps = []
    for i in range(N_CORES):
        r0 = i * 128  # this core's 128 rows of the per-part weight slices
        # Wqkv rows: q block rows r0:r0+128 (2 heads), k at d+..., v at 2d+...
        wq = Wqkv[r0:r0 + 128]            # (128, d)
        wk = Wqkv[d + r0:d + r0 + 128]
        wv = Wqkv[2 * d + r0:2 * d + r0 + 128]
        # lhsT layout: (d x e) -> blocks (n_dblk, 128, 384): [q|k|v]
        wT = np.concatenate([wq.T, wk.T, wv.T], axis=1)  # (d, 384)
        wT = wT.reshape(n_dblk, 128, 3 * 128).astype(ml_dtypes.bfloat16)
        # Wout columns r0:r0+128 -> rhs (128c x d)
        woT = Wout[:, r0:r0 + 128].T.astype(ml_dtypes.bfloat16)
        woT = np.ascontiguousarray(woT)
        in_maps.append({"xT": xT, "wqkvT": wT, "woutT": woT})
    return in_maps


_runner_cache = {}


def _make_runner(nc, n_cores):
    """Like bass2jax.run_bass_via_pjrt but with the jitted executable built
    once and cached, and output zero-buffers created on-device instead of
    being uploaded every call."""
    import jax
    import jax.numpy as jnp
    from jax.sharding import Mesh, PartitionSpec
    from jax.experimental.shard_map import shard_map
    import concourse.mybir as mybir
    from concourse import bass2jax

    bass2jax.install_neuronx_cc_hook()
    partition_name = (nc.partition_id_tensor.name
                      if nc.partition_id_tensor else None)
    in_names, out_names, out_avals = [], [], []
    for alloc in nc.m.functions[0].allocations:
        if not isinstance(alloc, mybir.MemoryLocationSet):
            continue
        name = alloc.memorylocations[0].name
        if alloc.kind == "ExternalInput":
            if name != partition_name:
                in_names.append(name)
        elif alloc.kind == "ExternalOutput":
            out_names.append(name)
            out_avals.append(jax.core.ShapedArray(
                tuple(alloc.tensor_shape), mybir.dt.np(alloc.dtype)))
    n_params = len(in_names)
    n_outs = len(out_names)
    bind_names = list(in_names) + list(out_names)
    if partition_name is not None:
        bind_names.append(partition_name)

    def _body(*args):
        operands = list(args)
        if partition_name is not None:
            operands.append(bass2jax.partition_id_tensor())
        outs = bass2jax._bass_exec_p.bind(
            *operands,
            out_avals=tuple(out_avals),
            in_names=tuple(bind_names),
            out_names=tuple(out_names),
            lowering_input_output_aliases=(),
            sim_require_finite=True,
            sim_require_nnan=True,
            nc=nc,
        )
        return tuple(outs)

    devices = jax.devices()[:n_cores]
    mesh = Mesh(np.array(devices), ("core",))
    sharded = jax.jit(
        shard_map(
            _body, mesh=mesh,
            in_specs=(PartitionSpec("core"),) * (n_params + n_outs),
            out_specs=(PartitionSpec("core"),) * n_outs,
            check_rep=False),
        donate_argnums=tuple(range(n_params, n_params + n_outs)),
        keep_unused=True)

    def run(in_maps):
        concat_in = [
            np.concatenate([np.asarray(m[name]) for m in in_maps], axis=0)
            for name in in_names]
        concat_zeros = [
            np.zeros((n_cores * a.shape[0], *a.shape[1:]), a.dtype)
            for a in out_avals]
        out_arrs = sharded(*concat_in, *concat_zeros)
        return [
            {name: np.asarray(out_arrs[i]).reshape(
                n_cores, *out_avals[i].shape)[c]
             for i, name in enumerate(out_names)}
            for c in range(n_cores)]

    return run


def kernel(x, Wqkv, Wout, _trace=False):
    b, s, d = x.shape
    nc = _get_kernel(b, s)
    in_maps = _prep_inputs(np.asarray(x), np.asarray(Wqkv), np.asarray(Wout))
    if _trace:
        from concourse.bass_utils import run_bass_kernel_spmd
        res = run_bass_kernel_spmd(nc, in_maps,
                                   core_ids=list(range(N_CORES)), trace=True)
        results = res.results
        kernel.last_results = res
    else:
        key = id(nc)
        if key not in _runner_cache:
            _runner_cache[key] = _make_runner(nc, N_CORES)
        results = _runner_cache[key](in_maps)
    acc = results[0]["partial_out"].astype(np.float32)
    for i in range(1, N_CORES):
        acc = acc + results[i]["partial_out"]
    return acc.reshape(b, s, d)



# revision 6
# speedup vs baseline: 1.1692x; 1.1692x over previous
"""Causal multi-head self-attention on 8 Trainium2 NeuronCores.

Sharding: head-parallel. Each of the 8 cores owns 2 of the 16 heads:
it computes Q/K/V for its heads (full sequence), runs causal flash
attention for them entirely on-chip, applies its slice of the output
projection, and writes a full-shape partial output. The host sums the
8 partials.

v2 schedule: the kernel is jointly PE- and ACT(exp)-bound, so the
emission order interleaves at kb-block granularity: QKV-projection and
output-projection matmuls are queued as "fillers" and dropped one or
two at a time between the score/AV matmuls of the attention inner
loop. The PE never idles (stays at max p-state) while the Scalar
engine streams exp calls; output projection runs inline per q-tile so
its DMA overlaps the whole kernel instead of forming a tail.

Layout:
  - x is cast to bf16 on host and staged transposed; one DMA per
    (batch, 512-token) tile loads all 8 d-blocks.
  - Q^T, K^T are (128 = [h0|h1] x 64) x t, the exact lhsT/rhs layout
    the transposed score matmuls need; score pairs dual-issue on the
    PE via row-disjoint tile_position quadrants.
  - exp runs on ScalarE straight out of PSUM (scale=1/8 fused), a
    single call per k-block covering both heads (3D AP on diagonals).
  - V is stored per (batch, kblock) as 129 columns [v_h0 | ones |
    v_h1]; the shared ones column makes both heads' AV matmuls emit
    the softmax denominator as an extra output row for free.
  - Causal masking: diagonal blocks are narrowed to the valid q range
    and the 128-column boundary gets a precomputed 0/1 triangle
    multiply after exp.
  - Normalization at AV eviction: reciprocal row broadcast over
    partitions (GpSimd) then one fused multiply PSUM->SBUF into ctx^T.
  - Output projection consumes ctx^T blocks as stationary operands so
    results land (t x e); one DMA per (batch, q-tile) writes them out.
"""

import numpy as np
import sys

for _p in ("/opt/trn_rl_repo", "/root/.axon_site/_ro/trn_rl_repo"):
    if _p not in sys.path:
        sys.path.append(_p)

import ml_dtypes

B = 2
S = 4096
D = 1024
H = 16
DH = 64
N_CORES = 8
HEADS_PER_CORE = H // N_CORES  # 2

_cache = {}


def _build(nc, b, s):
    import concourse.bass as bass
    import concourse.mybir as mybir
    from concourse.tile import TileContext
    from contextlib import ExitStack

    dt = mybir.dt
    AF = mybir.ActivationFunctionType
    ALU = mybir.AluOpType

    t_total = b * s          # 8192
    TT = 512                 # t tile (QKV free dim)
    n_dblk = D // 128        # 8
    QT = 512                 # q tile
    n_qt = s // QT           # per batch (8)
    KB = 128                 # k block
    n_kblk = s // KB         # 32
    scale = 1.0 / np.sqrt(DH)

    x_d = nc.dram_tensor("xT", [D, t_total], dt.bfloat16, kind="ExternalInput")
    wqkv_d = nc.dram_tensor("wqkvT", [n_dblk, 128, 3 * 128], dt.bfloat16,
                            kind="ExternalInput")
    wout_d = nc.dram_tensor("woutT", [128, D], dt.bfloat16, kind="ExternalInput")
    out_d = nc.dram_tensor("partial_out", [t_total, D], dt.bfloat16,
                           kind="ExternalOutput")

    with TileContext(nc) as tc, ExitStack() as ctx:
        const = ctx.enter_context(tc.tile_pool(name="const", bufs=1))
        wqkvT = const.tile([128, n_dblk, 3 * 128], dt.bfloat16, tag="wqkv")
        woutT = const.tile([128, D], dt.bfloat16, tag="wout")
        qT = const.tile([128, t_total], dt.bfloat16, tag="qT")
        kT = const.tile([128, t_total], dt.bfloat16, tag="kT")
        # V: per (batch, kblock) 130 cols [v_h0 | ones | v_h1 | ones]
        vst = const.tile([128, b, n_kblk, 2 * DH + 2], dt.bfloat16, tag="vst")
        ctxT = const.tile([128, t_total], dt.bfloat16, tag="ctxT")
        tri = const.tile([128, 128], dt.bfloat16, tag="tri")
        ident = const.tile([128, 128], dt.bfloat16, tag="ident")

        nc.sync.dma_start(wqkvT[:], wqkv_d.rearrange("k p e -> p k e"))
        nc.sync.dma_start(woutT[:], wout_d[:])

        # ones column of vst, the 0/1 lower-triangle mask (keep k<=q: in
        # (k=partition r, q=col c) space keep c >= r), and the identity
        # for the PE transpose of V.
        nc.vector.memset(vst[:, :, :, DH], 1.0)
        nc.vector.memset(vst[:, :, :, 2 * DH + 1], 1.0)
        nc.gpsimd.memset(tri[:], 1.0)
        nc.gpsimd.affine_select(
            tri[:], tri[:], pattern=[[1, 128]], compare_op=ALU.is_ge,
            fill=0.0, base=0, channel_multiplier=-1,
        )
        nc.gpsimd.affine_select(
            ident[:], tri[:], pattern=[[1, 128]], compare_op=ALU.is_equal,
            fill=0.0, base=0, channel_multiplier=-1,
        )

        # SBUF pools
        xt_pool = ctx.enter_context(tc.tile_pool(name="xt", bufs=4))
        pt_pool = ctx.enter_context(tc.tile_pool(name="pt", bufs=4))
        vt_pool = ctx.enter_context(tc.tile_pool(name="vt", bufs=2))
        ev_pool = ctx.enter_context(tc.tile_pool(name="ev", bufs=4))
        out_sb_pool = ctx.enter_context(tc.tile_pool(name="out_sb", bufs=2))
        # PSUM: 8 banks = scores 2x2 + o65 2 + skip(qkv/transpose/outproj) 2
        sc_ps = ctx.enter_context(tc.tile_pool(name="sc_ps", bufs=2, space="PSUM"))
        o65_ps = ctx.enter_context(tc.tile_pool(name="o65_ps", bufs=1, space="PSUM"))
        skip_ps = ctx.enter_context(tc.tile_pool(name="skip_ps", bufs=2,
                                                 space="PSUM"))

        xt_tiles = {}

        def xt_load(bb, tt):
            """One DMA: all 8 d-blocks of a (bb, tt) token tile."""
            xt = xt_pool.tile([128, n_dblk, TT], dt.bfloat16, tag="xt")
            t0 = bb * s + tt * TT
            nc.sync.dma_start(
                xt[:], x_d.rearrange("(k p) t -> p k t", p=128)[:, :, t0:t0 + TT])
            xt_tiles[(bb, tt)] = xt

        def qkv_fillers(bb, tt):
            """Emit QKV projection for (bb, tt) as a list of PE closures.

            q and k accumulate into the two skip-pool slots; v reuses
            q's slot after eviction, then 4 PE transposes scatter V into
            vst via two DVE copies each.
            """
            t0 = bb * s + tt * TT
            xt = xt_tiles.pop((bb, tt))
            state = {}

            def mk_qk(dd):
                def emit():
                    if dd == 0:
                        state["ps_q"] = skip_ps.tile([128, TT], dt.float32,
                                                     name="ps_q", tag="skip")
                        state["ps_k"] = skip_ps.tile([128, TT], dt.float32,
                                                     name="ps_k", tag="skip")
                    st = dict(start=(dd == 0), stop=(dd == n_dblk - 1))
                    nc.tensor.matmul(state["ps_q"][:], wqkvT[:, dd, 0:128],
                                     xt[:, dd, :], **st)
                    nc.tensor.matmul(state["ps_k"][:], wqkvT[:, dd, 128:256],
                                     xt[:, dd, :], **st)
                    if dd == n_dblk - 1:
                        nc.vector.tensor_copy(qT[:, t0:t0 + TT], state["ps_q"][:])
                        nc.vector.tensor_copy(kT[:, t0:t0 + TT], state["ps_k"][:])
                return emit

            def mk_v(dd):
                def emit():
                    if dd == 0:
                        state["ps_v"] = skip_ps.tile([128, TT], dt.float32,
                                                     name="ps_v", tag="skip")
                    st = dict(start=(dd == 0), stop=(dd == n_dblk - 1))
                    nc.tensor.matmul(state["ps_v"][:], wqkvT[:, dd, 256:384],
                                     xt[:, dd, :], **st)
                    if dd == n_dblk - 1:
                        vt = vt_pool.tile([128, TT], dt.bfloat16, tag="vt")
                        nc.vector.tensor_copy(vt[:], state["ps_v"][:])
                        state["vt"] = vt
                return emit

            def mk_tr(j):
                def emit():
                    ps_tv = skip_ps.tile([128, 128], dt.bfloat16, name="ps_tv",
                                         tag="skip")
                    nc.tensor.transpose(ps_tv[:],
                                        state["vt"][:, j * 128:(j + 1) * 128],
                                        ident[:])
                    kb = (tt * TT) // KB + j
                    nc.vector.tensor_copy(vst[:, bb, kb, 0:DH], ps_tv[:, 0:DH])
                    nc.vector.tensor_copy(vst[:, bb, kb, DH + 1:2 * DH + 1],
                                          ps_tv[:, DH:2 * DH])
                    # (col DH and col 2*DH+1 hold the ones columns)
                return emit

            return ([mk_qk(dd) for dd in range(n_dblk)]
                    + [mk_v(dd) for dd in range(n_dblk)]
                    + [mk_tr(j) for j in range(TT // 128)])

        def outproj_fillers(bb, qt):
            """Output projection for (bb, qt): 8 matmul closures + 1 DMA."""
            tq0 = bb * s + qt * QT
            state = {}

            def mk(i):
                tb, e = divmod(i, 2)

                def emit():
                    if i == 0:
                        state["ob"] = out_sb_pool.tile(
                            [128, QT // 128, D], dt.bfloat16, name="ob",
                            tag="ob")
                    ps = skip_ps.tile([128, 512], dt.float32, name="ps_op",
                                      tag="skip")
                    t0 = tq0 + tb * 128
                    nc.tensor.matmul(ps[:], ctxT[:, t0:t0 + 128],
                                     woutT[:, e * 512:(e + 1) * 512])
                    nc.vector.tensor_copy(
                        state["ob"][:, tb, e * 512:(e + 1) * 512], ps[:])
                    if i == 7:
                        nc.sync.dma_start(
                            out_d.rearrange("(u j p) e -> p u j e", p=128,
                                            j=QT // 128)[:, bb * n_qt + qt],
                            state["ob"][:])
                return emit

            return [mk(i) for i in range(8)]

        filler_q = []

        def pop_fillers(k):
            for _ in range(min(k, len(filler_q))):
                filler_q.pop(0)()

        def attention(bb, qt):
            """One q-tile of causal attention for both heads of batch bb.

            Emits scores(kb+1) before AV(kb) so the PE is never
            head-of-line blocked on exp(kb); fillers are popped between
            iterations to absorb the remaining ACT/PE rate mismatch.
            """
            tq0 = bb * s + qt * QT
            o65_h0 = o65_ps.tile([DH + 1, QT], dt.float32, tag="o65h0")
            o65_h1 = o65_ps.tile([DH + 1, QT], dt.float32, tag="o65h1")
            nkb = (qt + 1) * QT // KB
            # force-drain rate so filler_q empties by the end of this tile
            per_slot = max(1, -(-len(filler_q) // nkb))

            pts = {}

            def scores_exp(kb):
                tk0 = bb * s + kb * KB
                j = kb - qt * (QT // KB)  # >= 0 on the diagonal
                qc0 = max(j, 0) * KB      # first valid local q column
                w = QT - qc0
                ps_s = sc_ps.tile([128, 2, QT], dt.float32, tag="ps_s")
                nc.tensor.matmul(ps_s[:, 0, 0:w], kT[0:64, tk0:tk0 + KB],
                                 qT[0:64, tq0 + qc0:tq0 + QT],
                                 tile_position=(0, 0))
                nc.tensor.matmul(ps_s[:, 1, 0:w], kT[64:128, tk0:tk0 + KB],
                                 qT[64:128, tq0 + qc0:tq0 + QT],
                                 tile_position=(64, 0))
                pt = pt_pool.tile([128, 2, QT], dt.bfloat16, tag="pt")
                nc.scalar.activation(pt[:, :, 0:w], ps_s[:, :, 0:w],
                                     AF.Exp, scale=scale)
                if j >= 0:
                    for h in (0, 1):
                        nc.vector.tensor_tensor(
                            pt[:, h, 0:KB], pt[:, h, 0:KB], tri[:], ALU.mult)
                pts[kb] = (pt, qc0, w)

            def av(kb):
                pt, qc0, w = pts.pop(kb)
                st = dict(start=(kb == 0), stop=(kb == nkb - 1))
                nc.tensor.matmul(o65_h0[:, qc0:QT],
                                 vst[:, bb, kb, 0:DH + 1],
                                 pt[:, 0, 0:w], **st)
                nc.tensor.matmul(o65_h1[:, qc0:QT],
                                 vst[:, bb, kb, DH + 1:2 * DH + 2],
                                 pt[:, 1, 0:w], **st)

            scores_exp(0)
            for kb in range(nkb):
                if kb + 1 < nkb:
                    scores_exp(kb + 1)
                pop_fillers(per_slot)
                av(kb)

            # normalize: both heads produce [o(64); den] in PSUM
            for h, o65 in ((0, o65_h0), (1, o65_h1)):
                row = ev_pool.tile([1, QT], dt.float32, tag="row")
                rec = ev_pool.tile([1, QT], dt.float32, tag="rec")
                bc = ev_pool.tile([64, QT], dt.float32, tag="bc")
                nc.vector.tensor_copy(row[:], o65[DH:DH + 1, :])
                nc.vector.reciprocal_approx_fast(rec[:], row[:])
                nc.gpsimd.partition_broadcast(bc[:], rec[:])
                nc.vector.tensor_tensor(
                    ctxT[h * DH:(h + 1) * DH, tq0:tq0 + QT],
                    o65[0:DH, :], bc[:], ALU.mult)

        # ---- emission schedule ----
        for bb in range(b):
            xt_load(bb, 0)
        for bb in range(b):
            xt_load(bb, 1)
        for bb in range(b):
            for f in qkv_fillers(bb, 0):
                f()
        for qt in range(n_qt):
            for bb in range(b):
                if qt + 1 < n_qt:
                    if qt + 2 < n_qt and (bb, qt + 2) not in xt_tiles:
                        xt_load(bb, qt + 2)
                    filler_q.extend(qkv_fillers(bb, qt + 1))
                attention(bb, qt)
                filler_q.extend(outproj_fillers(bb, qt))
        for f in filler_q:
            f()
        filler_q.clear()

    return nc


def _get_kernel(b, s):
    key = (b, s)
    if key not in _cache:
        from concourse import bacc
        nc = bacc.Bacc()
        _build(nc, b, s)
        nc.finalize()
        _cache[key] = nc
    return _cache[key]


def _prep_inputs(x, Wqkv, Wout):
    """Host-side shard + transpose + bf16 cast. Returns list of in_maps."""
    b, s, d = x.shape
    xT = np.ascontiguousarray(
        x.reshape(b * s, d).astype(ml_dtypes.bfloat16).T)  # (d, b*s)
    n_dblk = d // 128
    in_maps = []
    for i in range(N_CORES):
        r0 = i * 128
        wq = Wqkv[r0:r0 + 128]            # (128, d)
        wk = Wqkv[d + r0:d + r0 + 128]
        wv = Wqkv[2 * d + r0:2 * d + r0 + 128]
        wT = np.concatenate([wq.T, wk.T, wv.T], axis=1)  # (d, 384)
        wT = wT.reshape(n_dblk, 128, 3 * 128).astype(ml_dtypes.bfloat16)
        woT = Wout[:, r0:r0 + 128].T.astype(ml_dtypes.bfloat16)
        woT = np.ascontiguousarray(woT)
        in_maps.append({"xT": xT, "wqkvT": wT, "woutT": woT})
    return in_maps


_runner_cache = {}


def _make_runner(nc, n_cores):
    """Like bass2jax.run_bass_via_pjrt but with the jitted executable built
    once and cached, and output zero-buffers created on-device instead of
    being uploaded every call."""
    import jax
    import jax.numpy as jnp
    from jax.sharding import Mesh, PartitionSpec
    from jax.experimental.shard_map import shard_map
    import concourse.mybir as mybir
    from concourse import bass2jax

    bass2jax.install_neuronx_cc_hook()
    partition_name = (nc.partition_id_tensor.name
                      if nc.partition_id_tensor else None)
    in_names, out_names, out_avals = [], [], []
    for alloc in nc.m.functions[0].allocations:
        if not isinstance(alloc, mybir.MemoryLocationSet):
            continue
        name = alloc.memorylocations[0].name
        if alloc.kind == "ExternalInput":
            if name != partition_name:
                in_names.append(name)
        elif alloc.kind == "ExternalOutput":
            out_names.append(name)
            out_avals.append(jax.core.ShapedArray(
                tuple(alloc.tensor_shape), mybir.dt.np(alloc.dtype)))
    n_params = len(in_names)
    n_outs = len(out_names)
    bind_names = list(in_names) + list(out_names)
    if partition_name is not None:
        bind_names.append(partition_name)

    def _body(*args):
        operands = list(args)
        if partition_name is not None:
            operands.append(bass2jax.partition_id_tensor())
        outs = bass2jax._bass_exec_p.bind(
            *operands,
            out_avals=tuple(out_avals),
            in_names=tuple(bind_names),
            out_names=tuple(out_names),
            lowering_input_output_aliases=(),
            sim_require_finite=True,
            sim_require_nnan=True,
            nc=nc,
        )
        return tuple(outs)

    devices = jax.devices()[:n_cores]
    mesh = Mesh(np.array(devices), ("core",))
    sharded = jax.jit(
        shard_map(
            _body, mesh=mesh,
            in_specs=(PartitionSpec("core"),) * (n_params + n_outs),
            out_specs=(PartitionSpec("core"),) * n_outs,
            check_rep=False),
        donate_argnums=tuple(range(n_params, n_params + n_outs)),
        keep_unused=True)

    def run(in_maps):
        concat_in = [
            np.concatenate([np.asarray(m[name]) for m in in_maps], axis=0)
            for name in in_names]
        concat_zeros = [
            np.zeros((n_cores * a.shape[0], *a.shape[1:]), a.dtype)
            for a in out_avals]
        out_arrs = sharded(*concat_in, *concat_zeros)
        return [
            {name: np.asarray(out_arrs[i]).reshape(
                n_cores, *out_avals[i].shape)[c]
             for i, name in enumerate(out_names)}
            for c in range(n_cores)]

    return run


def kernel(x, Wqkv, Wout, _trace=False):
    b, s, d = x.shape
    nc = _get_kernel(b, s)
    in_maps = _prep_inputs(np.asarray(x), np.asarray(Wqkv), np.asarray(Wout))
    if _trace:
        from concourse.bass_utils import run_bass_kernel_spmd
        res = run_bass_kernel_spmd(nc, in_maps,
                                   core_ids=list(range(N_CORES)), trace=True)
        results = res.results
        kernel.last_results = res
    else:
        key = id(nc)
        if key not in _runner_cache:
            _runner_cache[key] = _make_runner(nc, N_CORES)
        results = _runner_cache[key](in_maps)
    acc = results[0]["partial_out"].astype(np.float32)
    for i in range(1, N_CORES):
        acc = acc + results[i]["partial_out"]
    return acc.reshape(b, s, d)


# revision 8
# speedup vs baseline: 1.2871x; 1.1008x over previous
"""Causal multi-head self-attention on 8 Trainium2 NeuronCores.

Sharding: head-parallel. Each of the 8 cores owns 2 of the 16 heads:
it computes Q/K/V for its heads (full sequence), runs causal flash
attention for them entirely on-chip, applies its slice of the output
projection, and writes a full-shape partial output. The host sums the
8 partials.

v2 schedule: the kernel is jointly PE- and ACT(exp)-bound, so the
emission order interleaves at kb-block granularity: QKV-projection and
output-projection matmuls are queued as "fillers" and dropped one or
two at a time between the score/AV matmuls of the attention inner
loop. The PE never idles (stays at max p-state) while the Scalar
engine streams exp calls; output projection runs inline per q-tile so
its DMA overlaps the whole kernel instead of forming a tail.

Layout:
  - x is cast to bf16 on host and staged transposed; one DMA per
    (batch, 512-token) tile loads all 8 d-blocks.
  - Q^T, K^T are (128 = [h0|h1] x 64) x t, the exact lhsT/rhs layout
    the transposed score matmuls need; score pairs dual-issue on the
    PE via row-disjoint tile_position quadrants.
  - exp runs on ScalarE straight out of PSUM (scale=1/8 fused), a
    single call per k-block covering both heads (3D AP on diagonals).
  - V is stored per (batch, kblock) as 129 columns [v_h0 | ones |
    v_h1]; the shared ones column makes both heads' AV matmuls emit
    the softmax denominator as an extra output row for free.
  - Causal masking: diagonal blocks are narrowed to the valid q range
    and the 128-column boundary gets a precomputed 0/1 triangle
    multiply after exp.
  - Normalization at AV eviction: reciprocal row broadcast over
    partitions (GpSimd) then one fused multiply PSUM->SBUF into ctx^T.
  - Output projection consumes ctx^T blocks as stationary operands so
    results land (t x e); one DMA per (batch, q-tile) writes them out.
"""

import numpy as np
import sys

for _p in ("/opt/trn_rl_repo", "/root/.axon_site/_ro/trn_rl_repo"):
    if _p not in sys.path:
        sys.path.append(_p)

import ml_dtypes

B = 2
S = 4096
D = 1024
H = 16
DH = 64
N_CORES = 8
HEADS_PER_CORE = H // N_CORES  # 2

_cache = {}


def _build(nc, b, s):
    import concourse.bass as bass
    import concourse.mybir as mybir
    from concourse.tile import TileContext
    from contextlib import ExitStack

    dt = mybir.dt
    AF = mybir.ActivationFunctionType
    ALU = mybir.AluOpType

    t_total = b * s          # 8192
    TT = 512                 # t tile (QKV free dim)
    n_dblk = D // 128        # 8
    QT = 512                 # q tile
    n_qt = s // QT           # per batch (8)
    KB = 128                 # k block
    n_kblk = s // KB         # 32
    scale = 1.0 / np.sqrt(DH)

    x_d = nc.dram_tensor("xT", [D, t_total], dt.bfloat16, kind="ExternalInput")
    wqkv_d = nc.dram_tensor("wqkvT", [n_dblk, 128, 3 * 128], dt.bfloat16,
                            kind="ExternalInput")
    wout_d = nc.dram_tensor("woutT", [128, D], dt.bfloat16, kind="ExternalInput")
    out_d = nc.dram_tensor("partial_out", [t_total, D], dt.bfloat16,
                           kind="ExternalOutput")

    with TileContext(nc) as tc, ExitStack() as ctx:
        const = ctx.enter_context(tc.tile_pool(name="const", bufs=1))
        wqkvT = const.tile([128, n_dblk, 3 * 128], dt.bfloat16, tag="wqkv")
        woutT = const.tile([128, D], dt.bfloat16, tag="wout")
        qT = const.tile([128, t_total], dt.bfloat16, tag="qT")
        kT = const.tile([128, t_total], dt.bfloat16, tag="kT")
        # V: per (batch, kblock) 130 cols [v_h0 | ones | v_h1 | ones]
        vst = const.tile([128, b, n_kblk, 2 * DH + 2], dt.bfloat16, tag="vst")
        ctxT = const.tile([128, t_total], dt.bfloat16, tag="ctxT")
        tri = const.tile([128, 128], dt.bfloat16, tag="tri")
        ident = const.tile([128, 128], dt.bfloat16, tag="ident")

        nc.sync.dma_start(wqkvT[:], wqkv_d.rearrange("k p e -> p k e"))
        nc.sync.dma_start(woutT[:], wout_d[:])

        # ones column of vst, the 0/1 lower-triangle mask (keep k<=q: in
        # (k=partition r, q=col c) space keep c >= r), and the identity
        # for the PE transpose of V.
        nc.vector.memset(vst[:, :, :, DH], 1.0)
        nc.vector.memset(vst[:, :, :, 2 * DH + 1], 1.0)
        nc.gpsimd.memset(tri[:], 1.0)
        nc.gpsimd.affine_select(
            tri[:], tri[:], pattern=[[1, 128]], compare_op=ALU.is_ge,
            fill=0.0, base=0, channel_multiplier=-1,
        )
        nc.gpsimd.affine_select(
            ident[:], tri[:], pattern=[[1, 128]], compare_op=ALU.is_equal,
            fill=0.0, base=0, channel_multiplier=-1,
        )

        # SBUF pools
        xt_pool = ctx.enter_context(tc.tile_pool(name="xt", bufs=4))
        pt_pool = ctx.enter_context(tc.tile_pool(name="pt", bufs=4))
        vt_pool = ctx.enter_context(tc.tile_pool(name="vt", bufs=2))
        ev_pool = ctx.enter_context(tc.tile_pool(name="ev", bufs=4))
        out_sb_pool = ctx.enter_context(tc.tile_pool(name="out_sb", bufs=2))
        # PSUM: 8 banks = scores 2x2 + o65 2 + skip(qkv/transpose/outproj) 2
        sc_ps = ctx.enter_context(tc.tile_pool(name="sc_ps", bufs=2, space="PSUM"))
        o65_ps = ctx.enter_context(tc.tile_pool(name="o65_ps", bufs=1, space="PSUM"))
        skip_ps = ctx.enter_context(tc.tile_pool(name="skip_ps", bufs=2,
                                                 space="PSUM"))

        xt_tiles = {}

        def xt_load(bb, tt):
            """One DMA: all 8 d-blocks of a (bb, tt) token tile."""
            xt = xt_pool.tile([128, n_dblk, TT], dt.bfloat16, tag="xt")
            t0 = bb * s + tt * TT
            nc.sync.dma_start(
                xt[:], x_d.rearrange("(k p) t -> p k t", p=128)[:, :, t0:t0 + TT])
            xt_tiles[(bb, tt)] = xt

        def qkv_fillers(bb, tt):
            """Emit QKV projection for (bb, tt) as a list of PE closures.

            q and k accumulate into the two skip-pool slots; v reuses
            q's slot after eviction, then 4 PE transposes scatter V into
            vst via two DVE copies each.
            """
            t0 = bb * s + tt * TT
            xt = xt_tiles.pop((bb, tt))
            state = {}

            def mk_qk(dd):
                def emit():
                    if dd == 0:
                        state["ps_q"] = skip_ps.tile([128, TT], dt.float32,
                                                     name="ps_q", tag="skip")
                        state["ps_k"] = skip_ps.tile([128, TT], dt.float32,
                                                     name="ps_k", tag="skip")
                    st = dict(start=(dd == 0), stop=(dd == n_dblk - 1))
                    nc.tensor.matmul(state["ps_q"][:], wqkvT[:, dd, 0:128],
                                     xt[:, dd, :], **st)
                    nc.tensor.matmul(state["ps_k"][:], wqkvT[:, dd, 128:256],
                                     xt[:, dd, :], **st)
                    if dd == n_dblk - 1:
                        nc.vector.tensor_copy(qT[:, t0:t0 + TT], state["ps_q"][:])
                        nc.vector.tensor_copy(kT[:, t0:t0 + TT], state["ps_k"][:])
                return emit

            def mk_v(dd):
                def emit():
                    if dd == 0:
                        state["ps_v"] = skip_ps.tile([128, TT], dt.float32,
                                                     name="ps_v", tag="skip")
                    st = dict(start=(dd == 0), stop=(dd == n_dblk - 1))
                    nc.tensor.matmul(state["ps_v"][:], wqkvT[:, dd, 256:384],
                                     xt[:, dd, :], **st)
                    if dd == n_dblk - 1:
                        vt = vt_pool.tile([128, TT], dt.bfloat16, tag="vt")
                        nc.vector.tensor_copy(vt[:], state["ps_v"][:])
                        state["vt"] = vt
                return emit

            def mk_tr(j):
                def emit():
                    ps_tv = skip_ps.tile([128, 128], dt.bfloat16, name="ps_tv",
                                         tag="skip")
                    nc.tensor.transpose(ps_tv[:],
                                        state["vt"][:, j * 128:(j + 1) * 128],
                                        ident[:])
                    kb = (tt * TT) // KB + j
                    nc.vector.tensor_copy(vst[:, bb, kb, 0:DH], ps_tv[:, 0:DH])
                    nc.vector.tensor_copy(vst[:, bb, kb, DH + 1:2 * DH + 1],
                                          ps_tv[:, DH:2 * DH])
                    # (col DH and col 2*DH+1 hold the ones columns)
                return emit

            return ([mk_qk(dd) for dd in range(n_dblk)]
                    + [mk_v(dd) for dd in range(n_dblk)]
                    + [mk_tr(j) for j in range(TT // 128)])

        def outproj_fillers(bb, qt):
            """Output projection for (bb, qt): 8 matmul closures + 1 DMA."""
            tq0 = bb * s + qt * QT
            state = {}

            def mk(i):
                tb, e = divmod(i, 2)

                def emit():
                    if i == 0:
                        state["ob"] = out_sb_pool.tile(
                            [128, QT // 128, D], dt.bfloat16, name="ob",
                            tag="ob")
                    ps = skip_ps.tile([128, 512], dt.float32, name="ps_op",
                                      tag="skip")
                    t0 = tq0 + tb * 128
                    nc.tensor.matmul(ps[:], ctxT[:, t0:t0 + 128],
                                     woutT[:, e * 512:(e + 1) * 512])
                    nc.vector.tensor_copy(
                        state["ob"][:, tb, e * 512:(e + 1) * 512], ps[:])
                    if i == 7:
                        nc.sync.dma_start(
                            out_d.rearrange("(u j p) e -> p u j e", p=128,
                                            j=QT // 128)[:, bb * n_qt + qt],
                            state["ob"][:])
                return emit

            return [mk(i) for i in range(8)]

        urgent_q = []
        lazy_q = []

        def pop_fillers(n_urgent, n_lazy):
            for _ in range(min(n_urgent, len(urgent_q))):
                urgent_q.pop(0)()
            for _ in range(min(n_lazy, len(lazy_q))):
                lazy_q.pop(0)()

        def attention(bb, qt):
            """One q-tile of causal attention for both heads of batch bb.

            Emits scores(kb+1) before AV(kb) so the PE is never
            head-of-line blocked on exp(kb); fillers are popped between
            iterations to absorb the remaining ACT/PE rate mismatch.
            """
            tq0 = bb * s + qt * QT
            o65_h0 = o65_ps.tile([DH + 1, QT], dt.float32, tag="o65h0")
            o65_h1 = o65_ps.tile([DH + 1, QT], dt.float32, tag="o65h1")
            nkb = (qt + 1) * QT // KB
            # urgent fillers (next tile's QKV) must drain within this
            # attention; lazy fillers (outproj) pace uniformly over the
            # remaining kb slots of the whole kernel so the late q-tiles
            # (which have no QKV work left) still get PE filler.
            per_slot_u = max(1, -(-len(urgent_q) // nkb))
            rem_slots = self_rem_slots[0]
            per_slot_l = max(1, -(-len(lazy_q) // max(rem_slots, 1))) \
                if lazy_q else 0

            pts = {}

            def scores_exp(kb):
                tk0 = bb * s + kb * KB
                j = kb - qt * (QT // KB)  # >= 0 on the diagonal
                qc0 = max(j, 0) * KB      # first valid local q column
                w = QT - qc0
                ps_s = sc_ps.tile([128, 2, QT], dt.float32, tag="ps_s")
                nc.tensor.matmul(ps_s[:, 0, 0:w], kT[0:64, tk0:tk0 + KB],
                                 qT[0:64, tq0 + qc0:tq0 + QT],
                                 tile_position=(0, 0))
                nc.tensor.matmul(ps_s[:, 1, 0:w], kT[64:128, tk0:tk0 + KB],
                                 qT[64:128, tq0 + qc0:tq0 + QT],
                                 tile_position=(64, 0))
                pt = pt_pool.tile([128, 2, QT], dt.bfloat16, tag="pt")
                nc.scalar.activation(pt[:, :, 0:w], ps_s[:, :, 0:w],
                                     AF.Exp, scale=scale)
                if j >= 0:
                    for h in (0, 1):
                        nc.vector.tensor_tensor(
                            pt[:, h, 0:KB], pt[:, h, 0:KB], tri[:], ALU.mult)
                pts[kb] = (pt, qc0, w)

            def av(kb):
                pt, qc0, w = pts.pop(kb)
                st = dict(start=(kb == 0), stop=(kb == nkb - 1))
                nc.tensor.matmul(o65_h0[:, qc0:QT],
                                 vst[:, bb, kb, 0:DH + 1],
                                 pt[:, 0, 0:w], **st)
                nc.tensor.matmul(o65_h1[:, qc0:QT],
                                 vst[:, bb, kb, DH + 1:2 * DH + 2],
                                 pt[:, 1, 0:w], **st)

            scores_exp(0)
            for kb in range(nkb):
                if kb + 1 < nkb:
                    scores_exp(kb + 1)
                pop_fillers(per_slot_u, per_slot_l)
                self_rem_slots[0] -= 1
                av(kb)

            # normalize: both heads produce [o(64); den] in PSUM
            for h, o65 in ((0, o65_h0), (1, o65_h1)):
                row = ev_pool.tile([1, QT], dt.float32, tag="row")
                rec = ev_pool.tile([1, QT], dt.float32, tag="rec")
                bc = ev_pool.tile([64, QT], dt.float32, tag="bc")
                nc.vector.tensor_copy(row[:], o65[DH:DH + 1, :])
                nc.vector.reciprocal_approx_fast(rec[:], row[:])
                nc.gpsimd.partition_broadcast(bc[:], rec[:])
                nc.vector.tensor_tensor(
                    ctxT[h * DH:(h + 1) * DH, tq0:tq0 + QT],
                    o65[0:DH, :], bc[:], ALU.mult)

        # ---- emission schedule ----
        # attention order A_i = (i%b, i//b); qkv unit Q_i matches; Q_{i+1}
        # is emitted as urgent filler during A_i.
        steps = [(i % b, i // b) for i in range(b * n_qt)]
        self_rem_slots = [sum((qt + 1) * QT // KB for _, qt in steps)]
        xt_load(*steps[0])
        xt_load(*steps[1])
        xt_load(*steps[2])
        for f in qkv_fillers(*steps[0]):
            f()
        for i, (bb, qt) in enumerate(steps):
            if i + 3 < len(steps):
                xt_load(*steps[i + 3])
            if i + 1 < len(steps):
                urgent_q.extend(qkv_fillers(*steps[i + 1]))
            attention(bb, qt)
            lazy_q.extend(outproj_fillers(bb, qt))
        for f in urgent_q + lazy_q:
            f()

    return nc


def _get_kernel(b, s):
    key = (b, s)
    if key not in _cache:
        from concourse import bacc
        nc = bacc.Bacc()
        _build(nc, b, s)
        nc.finalize()
        _cache[key] = nc
    return _cache[key]


def _prep_inputs(x, Wqkv, Wout):
    """Host-side shard + transpose + bf16 cast. Returns list of in_maps."""
    b, s, d = x.shape
    xT = np.ascontiguousarray(
        x.reshape(b * s, d).astype(ml_dtypes.bfloat16).T)  # (d, b*s)
    n_dblk = d // 128
    in_maps = []
    for i in range(N_CORES):
        r0 = i * 128
        wq = Wqkv[r0:r0 + 128]            # (128, d)
        wk = Wqkv[d + r0:d + r0 + 128]
        wv = Wqkv[2 * d + r0:2 * d + r0 + 128]
        wT = np.concatenate([wq.T, wk.T, wv.T], axis=1)  # (d, 384)
        wT = wT.reshape(n_dblk, 128, 3 * 128).astype(ml_dtypes.bfloat16)
        woT = Wout[:, r0:r0 + 128].T.astype(ml_dtypes.bfloat16)
        woT = np.ascontiguousarray(woT)
        in_maps.append({"xT": xT, "wqkvT": wT, "woutT": woT})
    return in_maps


_runner_cache = {}


def _make_runner(nc, n_cores):
    """Like bass2jax.run_bass_via_pjrt but with the jitted executable built
    once and cached, and output zero-buffers created on-device instead of
    being uploaded every call."""
    import jax
    import jax.numpy as jnp
    from jax.sharding import Mesh, PartitionSpec
    from jax.experimental.shard_map import shard_map
    import concourse.mybir as mybir
    from concourse import bass2jax

    bass2jax.install_neuronx_cc_hook()
    partition_name = (nc.partition_id_tensor.name
                      if nc.partition_id_tensor else None)
    in_names, out_names, out_avals = [], [], []
    for alloc in nc.m.functions[0].allocations:
        if not isinstance(alloc, mybir.MemoryLocationSet):
            continue
        name = alloc.memorylocations[0].name
        if alloc.kind == "ExternalInput":
            if name != partition_name:
                in_names.append(name)
        elif alloc.kind == "ExternalOutput":
            out_names.append(name)
            out_avals.append(jax.core.ShapedArray(
                tuple(alloc.tensor_shape), mybir.dt.np(alloc.dtype)))
    n_params = len(in_names)
    n_outs = len(out_names)
    bind_names = list(in_names) + list(out_names)
    if partition_name is not None:
        bind_names.append(partition_name)

    def _body(*args):
        operands = list(args)
        if partition_name is not None:
            operands.append(bass2jax.partition_id_tensor())
        outs = bass2jax._bass_exec_p.bind(
            *operands,
            out_avals=tuple(out_avals),
            in_names=tuple(bind_names),
            out_names=tuple(out_names),
            lowering_input_output_aliases=(),
            sim_require_finite=True,
            sim_require_nnan=True,
            nc=nc,
        )
        return tuple(outs)

    devices = jax.devices()[:n_cores]
    mesh = Mesh(np.array(devices), ("core",))
    sharded = jax.jit(
        shard_map(
            _body, mesh=mesh,
            in_specs=(PartitionSpec("core"),) * (n_params + n_outs),
            out_specs=(PartitionSpec("core"),) * n_outs,
            check_rep=False),
        donate_argnums=tuple(range(n_params, n_params + n_outs)),
        keep_unused=True)

    def run(in_maps):
        concat_in = [
            np.concatenate([np.asarray(m[name]) for m in in_maps], axis=0)
            for name in in_names]
        concat_zeros = [
            np.zeros((n_cores * a.shape[0], *a.shape[1:]), a.dtype)
            for a in out_avals]
        out_arrs = sharded(*concat_in, *concat_zeros)
        return [
            {name: np.asarray(out_arrs[i]).reshape(
                n_cores, *out_avals[i].shape)[c]
             for i, name in enumerate(out_names)}
            for c in range(n_cores)]

    return run


def kernel(x, Wqkv, Wout, _trace=False):
    b, s, d = x.shape
    nc = _get_kernel(b, s)
    in_maps = _prep_inputs(np.asarray(x), np.asarray(Wqkv), np.asarray(Wout))
    if _trace:
        from concourse.bass_utils import run_bass_kernel_spmd
        res = run_bass_kernel_spmd(nc, in_maps,
                                   core_ids=list(range(N_CORES)), trace=True)
        results = res.results
        kernel.last_results = res
    else:
        key = id(nc)
        if key not in _runner_cache:
            _runner_cache[key] = _make_runner(nc, N_CORES)
        results = _runner_cache[key](in_maps)
    acc = results[0]["partial_out"].astype(np.float32)
    for i in range(1, N_CORES):
        acc = acc + results[i]["partial_out"]
    return acc.reshape(b, s, d)
